# revision 3
# baseline (speedup 1.0000x reference)
"""Bilateral blur (kornia bilateral_blur, kernel 7x7, sigma_color=10,
sigma_space=(21,21), border reflect, L1 color distance) for a batch of
8 RGB 512x512 images, on 8 Trainium2 NeuronCores.

kernel(img) takes the FULL (8, 3, 512, 512) float32 batch and returns the
FULL (8, 3, 512, 512) float32 result. The batch is sharded one image per
NeuronCore (pure data parallelism); each core runs an identical Bass/Tile
kernel built here:

  - host pads each image to (3, 518, 518) reflect and casts to bf16
  - partition dim = 128 output rows (4 row-tiles per image)
  - all 7 row-shifted copies of the padded rows live in one SBUF tile
    [128, 7, 3, 520], loaded twice at x-offsets 0/1 ("phases") so every
    window x-shift is a 4-byte-aligned bf16 slice (keeps DVE 2x mode)
  - the 49 window offsets are processed as 24 mirror PAIRS (k, 48-k) plus
    the center: mirrored offsets share the same spatial weight, so the
    whole chain runs at doubled free-dim (half the instruction overhead,
    one Exp bias per pair); the center offset has w == space[3,3] exactly
    and reduces to three identity-matmuls with a pre-scaled identity.
  - per pair:
      s   = P - C                        (DVE tensor_tensor, bf16 2x)
      a   = |s|                          (ACT Abs for most pairs - ACT has
                                          slack; DVE int16 sign-mask for
                                          the rest, balancing the engines)
      d   = a_r + a_g + a_b              (DVE adds)
      u   = d^2 ; w = exp(g*u + ln s_k)  (ACT Square + Exp, scale/bias
                                          folded into the Exp affine)
      t   = w * P                        (DVE, w broadcast over channels)
      acc += t_0, t_1 ; den += w_0+w_1   (PE identity-matmuls into PSUM,
                                          exact fp32 accumulation)
  - epilogue: out = acc / (den + s_24)   (DVE reciprocal + multiply, fp32)
"""

import numpy as np
import ml_dtypes

import concourse.bass as bass
import concourse.bacc as bacc
import concourse.mybir as mybir
import concourse.tile as tile
from concourse.bass_utils import run_bass_kernel_spmd

KS = 7
PAD = 3
SIGMA_COLOR = 10.0
SIGMA_SPACE = 21.0
B, CH, H, W = 8, 3, 512, 512
PW = W + 2 * PAD  # 518
GAMMA = -0.5 / (SIGMA_COLOR**2)
N_CORES = 8
NPAIRS = 24
# fraction of mirror-pairs whose |s| runs on ACT instead of DVE
ABS_ACT_FRAC = 0.92


def _gauss1d(ks, sigma):
    x = np.arange(ks, dtype=np.float64) - ks // 2
    g = np.exp(-0.5 * (x / sigma) ** 2)
    return g / g.sum()


_SPACE = np.outer(_gauss1d(KS, SIGMA_SPACE), _gauss1d(KS, SIGMA_SPACE))


def _build():
    DT = mybir.dt.bfloat16
    nphase = 2

    nc = bacc.Bacc("TRN2", target_bir_lowering=False, debug=False,
                   num_devices=N_CORES)
    pad_d = nc.dram_tensor("pad", [CH, H + 2 * PAD, PW], DT,
                           kind="ExternalInput")
    id_d = nc.dram_tensor("ident", [128, 128], DT, kind="ExternalInput")
    id2_d = nc.dram_tensor("ident2", [128, 128], DT, kind="ExternalInput")
    bias_d = nc.dram_tensor("bias49", [KS * KS], mybir.dt.float32,
                            kind="ExternalInput")
    out_d = nc.dram_tensor("out", [CH, H, W], mybir.dt.float32,
                           kind="ExternalOutput")

    with tile.TileContext(nc) as tc:
        with (
            tc.tile_pool(name="consts", bufs=1) as consts,
            tc.tile_pool(name="tin", bufs=2) as tin,
            tc.tile_pool(name="work", bufs=5) as work,
            tc.tile_pool(name="big", bufs=3) as big,
            tc.tile_pool(name="outp", bufs=2) as outp,
            tc.tile_pool(name="psum", bufs=2, space="PSUM") as psum,
        ):
            ident = consts.tile([128, 128], DT)
            nc.sync.dma_start(out=ident[:], in_=id_d.ap())
            ident2 = consts.tile([128, 128], DT)
            nc.sync.dma_start(out=ident2[:], in_=id2_d.ap())
            biases = consts.tile([128, KS * KS], mybir.dt.float32)
            bsrc = bias_d.ap()
            bsrc_b = bass.AP(
                tensor=bsrc.tensor, offset=bsrc.offset,
                ap=[[0, 128], bsrc.ap[0]],
            )
            nc.sync.dma_start(out=biases[:], in_=bsrc_b)

            for yt in range(H // 128):
                y0 = 128 * yt
                Tall = {}
                for ph in range(nphase):
                    tt = tin.tile([128, KS, 3, 520], DT, tag=f"Tall{ph}")
                    Tall[ph] = tt
                    for i in range(KS):
                        xl = PW - ph
                        src = pad_d.ap()[:, y0 + i : y0 + i + 128, ph:PW]
                        nc.sync.dma_start(
                            out=tt[:, i, :, 0:xl], in_=src.transpose([1, 0, 2])
                        )

                def pslice(i, j):
                    ph = j % 2
                    e0 = j - ph
                    return Tall[ph][:, i, :, e0 : e0 + 512]

                def pairslice(k):
                    # [128, 2, 3, 512] covering offsets k and 48-k
                    i, j = divmod(k, KS)
                    s0 = pslice(i, j)
                    s1 = pslice(6 - i, 6 - j)
                    step = s1.offset - s0.offset
                    return bass.AP(
                        tensor=s0.tensor, offset=s0.offset,
                        ap=[s0.ap[0], [step, 2], s0.ap[1], s0.ap[2]],
                    )

                C = pslice(PAD, PAD)
                C2 = C.unsqueeze(1).broadcast_to([128, 2, 3, 512])

                acc = psum.tile([128, 3, 512], mybir.dt.float32, tag="acc")
                den = psum.tile([128, 512], mybir.dt.float32, tag="den")

                # center offset: acc += space[3,3] * C via pre-scaled identity
                for c in range(3):
                    nc.tensor.matmul(
                        acc[:, c, :], ident2[:], C[:, c, :],
                        start=True, stop=False, skip_group_check=True,
                    )

                for kk in range(NPAIRS):
                    k = kk
                    P2 = pairslice(k)
                    s2 = big.tile([128, 2, 3, 512], DT, tag="s2")
                    nc.vector.tensor_sub(s2[:], P2, C2)
                    if kk < ABS_ACT_FRAC * NPAIRS:
                        a2 = big.tile([128, 2, 3, 512], DT, tag="a2")
                        nc.scalar.activation(
                            a2[:], s2[:], mybir.ActivationFunctionType.Abs
                        )
                    else:
                        nc.vector.tensor_scalar(
                            s2[:].bitcast(mybir.dt.int16),
                            s2[:].bitcast(mybir.dt.int16),
                            0x7FFF, None, mybir.AluOpType.bitwise_and,
                        )
                        a2 = s2
                    d2 = work.tile([128, 2, 512], DT, tag="d2")
                    nc.vector.tensor_add(d2[:], a2[:, :, 0, :], a2[:, :, 1, :])
                    nc.vector.tensor_add(d2[:], d2[:], a2[:, :, 2, :])
                    u2 = work.tile([128, 2, 512], DT, tag="u2")
                    nc.scalar.activation(
                        u2[:], d2[:], mybir.ActivationFunctionType.Square
                    )
                    w2 = work.tile([128, 2, 512], DT, tag="w2")
                    nc.scalar.activation(
                        w2[:], u2[:], mybir.ActivationFunctionType.Exp,
                        bias=biases[:, k : k + 1], scale=GAMMA,
                    )
                    t2 = big.tile([128, 2, 3, 512], DT, tag="t2")
                    w2b = w2[:].unsqueeze(2).broadcast_to([128, 2, 3, 512])
                    nc.vector.tensor_mul(t2[:], P2, w2b)
                    sp = kk == NPAIRS - 1
                    for p in range(2):
                        for c in range(3):
                            nc.tensor.matmul(
                                acc[:, c, :], ident[:], t2[:, p, c, :],
                                start=False, stop=(sp and p == 1),
                                skip_group_check=True,
                            )
                        nc.tensor.matmul(
                            den[:], ident[:], w2[:, p, :],
                            start=(kk == 0 and p == 0), stop=(sp and p == 1),
                            skip_group_check=True,
                        )

                # r = 1/(den + s24) via one Newton step from y1 = 2 - dn:
                # dn is within ~3% of 1 so the result is good to ~5e-7 rel.
                r = outp.tile([128, 512], mybir.dt.float32, tag="r")
                dn = outp.tile([128, 512], mybir.dt.float32, tag="dn")
                y1 = outp.tile([128, 512], mybir.dt.float32, tag="y1")
                e1 = outp.tile([128, 512], mybir.dt.float32, tag="e1")
                nc.vector.tensor_scalar(
                    dn[:], den[:], float(_SPACE[3, 3]), None,
                    mybir.AluOpType.add,
                )
                nc.vector.tensor_scalar(
                    y1[:], dn[:], -1.0, 2.0, mybir.AluOpType.mult,
                    mybir.AluOpType.add,
                )
                nc.vector.tensor_mul(e1[:], dn[:], y1[:])
                nc.vector.tensor_scalar(
                    e1[:], e1[:], -1.0, 2.0, mybir.AluOpType.mult,
                    mybir.AluOpType.add,
                )
                nc.vector.tensor_mul(r[:], e1[:], y1[:])
                o = outp.tile([128, 3, 512], mybir.dt.float32, tag="o")
                rb = r[:].unsqueeze(1).broadcast_to([128, 3, 512])
                nc.vector.tensor_mul(o[:], acc[:], rb)
                nc.sync.dma_start(
                    out=out_d.ap()[:, y0 : y0 + 128, :].transpose([1, 0, 2]),
                    in_=o[:],
                )

    nc.compile()
    return nc


_NC_CACHE = {}


def _get_nc():
    if "nc" not in _NC_CACHE:
        _NC_CACHE["nc"] = _build()
    return _NC_CACHE["nc"]


def _host_inputs(img_core: np.ndarray):
    p = np.pad(img_core, ((0, 0), (PAD, PAD), (PAD, PAD)), mode="reflect")
    return {
        "pad": np.ascontiguousarray(p.astype(ml_dtypes.bfloat16)),
        "ident": np.eye(128, dtype=np.float32).astype(ml_dtypes.bfloat16),
        "ident2": (np.eye(128, dtype=np.float32) * float(_SPACE[3, 3])
                   ).astype(ml_dtypes.bfloat16),
        "bias49": np.log(_SPACE.reshape(-1)).astype(np.float32),
    }


def kernel(img: np.ndarray) -> np.ndarray:
    """img: (8, 3, 512, 512) float32 -> (8, 3, 512, 512) float32."""
    img = np.asarray(img, dtype=np.float32)
    assert img.shape == (B, CH, H, W), img.shape

    nc = _get_nc()
    in_maps = [_host_inputs(img[b]) for b in range(B)]
    res = run_bass_kernel_spmd(nc, in_maps, core_ids=list(range(N_CORES)))
    out = np.stack([res.results[b]["out"] for b in range(B)], axis=0)
    return out.astype(np.float32)


# revision 4
# speedup vs baseline: 1.1270x; 1.1270x over previous
"""Bilateral blur (kornia bilateral_blur, kernel 7x7, sigma_color=10,
sigma_space=(21,21), border reflect, L1 color distance) for a batch of
8 RGB 512x512 images, on 8 Trainium2 NeuronCores.

kernel(img) takes the FULL (8, 3, 512, 512) float32 batch and returns the
FULL (8, 3, 512, 512) float32 result. The batch is sharded one image per
NeuronCore (pure data parallelism); each core runs an identical Bass/Tile
kernel built here:

  - host pads each image to (3, 518, 518) reflect and casts to bf16
  - partition dim = 128 output rows (4 row-tiles per image)
  - all 7 row-shifted copies of the padded rows live in one SBUF tile
    [128, 7, 3, 520], loaded twice at x-offsets 0/1 ("phases") so every
    window x-shift is a 4-byte-aligned bf16 slice (keeps DVE 2x mode)
  - the 49 window offsets are processed as 24 mirror PAIRS (k, 48-k) plus
    the center: mirrored offsets share the same spatial weight, so the
    whole chain runs at doubled free-dim (half the instruction overhead,
    one Exp bias per pair); the center offset has w == space[3,3] exactly
    and reduces to three identity-matmuls with a pre-scaled identity.
  - per pair:
      s   = P - C                        (DVE tensor_tensor, bf16 2x)
      a   = |s|                          (ACT Abs for most pairs - ACT has
                                          slack; DVE int16 sign-mask for
                                          the rest, balancing the engines)
      d   = a_r + a_g + a_b              (PE identity-matmuls into PSUM)
      u   = d^2 ; w = exp(g*u + ln s_k)  (ACT Square + Exp, scale/bias
                                          folded into the Exp affine)
      t   = w * P                        (DVE, w broadcast over channels)
      acc += t_0, t_1 ; den += w_0+w_1   (PE identity-matmuls into PSUM,
                                          exact fp32 accumulation)
  - epilogue: out = acc / (den + s_24)   (DVE reciprocal + multiply, fp32)
"""

import numpy as np
import ml_dtypes

import concourse.bass as bass
import concourse.bacc as bacc
import concourse.mybir as mybir
import concourse.tile as tile
from concourse.bass_utils import run_bass_kernel_spmd

KS = 7
PAD = 3
SIGMA_COLOR = 10.0
SIGMA_SPACE = 21.0
B, CH, H, W = 8, 3, 512, 512
PW = W + 2 * PAD  # 518
GAMMA = -0.5 / (SIGMA_COLOR**2)
N_CORES = 8
NPAIRS = 24
# fraction of mirror-pairs whose |s| runs on ACT instead of DVE
ABS_ACT_FRAC = 0.35


def _gauss1d(ks, sigma):
    x = np.arange(ks, dtype=np.float64) - ks // 2
    g = np.exp(-0.5 * (x / sigma) ** 2)
    return g / g.sum()


_SPACE = np.outer(_gauss1d(KS, SIGMA_SPACE), _gauss1d(KS, SIGMA_SPACE))


def _build():
    DT = mybir.dt.bfloat16
    nphase = 2

    nc = bacc.Bacc("TRN2", target_bir_lowering=False, debug=False,
                   num_devices=N_CORES)
    pad_d = nc.dram_tensor("pad", [CH, H + 2 * PAD, PW], DT,
                           kind="ExternalInput")
    id_d = nc.dram_tensor("ident", [128, 128], DT, kind="ExternalInput")
    id2_d = nc.dram_tensor("ident2", [128, 128], DT, kind="ExternalInput")
    bias_d = nc.dram_tensor("bias49", [KS * KS], mybir.dt.float32,
                            kind="ExternalInput")
    out_d = nc.dram_tensor("out", [CH, H, W], mybir.dt.float32,
                           kind="ExternalOutput")

    with tile.TileContext(nc) as tc:
        with (
            tc.tile_pool(name="consts", bufs=1) as consts,
            tc.tile_pool(name="tin", bufs=2) as tin,
            tc.tile_pool(name="work", bufs=5) as work,
            tc.tile_pool(name="big", bufs=3) as big,
            tc.tile_pool(name="outp", bufs=2) as outp,
            tc.tile_pool(name="psum", bufs=1, space="PSUM") as psum,
            tc.tile_pool(name="dpsum", bufs=2, space="PSUM") as dpsum,
        ):
            ident = consts.tile([128, 128], DT)
            nc.sync.dma_start(out=ident[:], in_=id_d.ap())
            ident2 = consts.tile([128, 128], DT)
            nc.sync.dma_start(out=ident2[:], in_=id2_d.ap())
            biases = consts.tile([128, KS * KS], mybir.dt.float32)
            bsrc = bias_d.ap()
            bsrc_b = bass.AP(
                tensor=bsrc.tensor, offset=bsrc.offset,
                ap=[[0, 128], bsrc.ap[0]],
            )
            nc.sync.dma_start(out=biases[:], in_=bsrc_b)

            for yt in range(H // 128):
                y0 = 128 * yt
                Tall = {}
                for ph in range(nphase):
                    tt = tin.tile([128, KS, 3, 520], DT, tag=f"Tall{ph}")
                    Tall[ph] = tt
                    for i in range(KS):
                        xl = PW - ph
                        src = pad_d.ap()[:, y0 + i : y0 + i + 128, ph:PW]
                        nc.sync.dma_start(
                            out=tt[:, i, :, 0:xl], in_=src.transpose([1, 0, 2])
                        )

                def pslice(i, j):
                    ph = j % 2
                    e0 = j - ph
                    return Tall[ph][:, i, :, e0 : e0 + 512]

                def pairslice(k):
                    # [128, 2, 3, 512] covering offsets k and 48-k
                    i, j = divmod(k, KS)
                    s0 = pslice(i, j)
                    s1 = pslice(6 - i, 6 - j)
                    step = s1.offset - s0.offset
                    return bass.AP(
                        tensor=s0.tensor, offset=s0.offset,
                        ap=[s0.ap[0], [step, 2], s0.ap[1], s0.ap[2]],
                    )

                C = pslice(PAD, PAD)
                C2 = C.unsqueeze(1).broadcast_to([128, 2, 3, 512])

                acc = psum.tile([128, 3, 512], mybir.dt.float32, tag="acc")
                den = psum.tile([128, 512], mybir.dt.float32, tag="den")

                # center offset: acc += space[3,3] * C via pre-scaled identity
                for c in range(3):
                    nc.tensor.matmul(
                        acc[:, c, :], ident2[:], C[:, c, :],
                        start=True, stop=False, skip_group_check=True,
                    )

                for kk in range(NPAIRS):
                    k = kk
                    P2 = pairslice(k)
                    s2 = big.tile([128, 2, 3, 512], DT, tag="s2")
                    nc.vector.tensor_sub(s2[:], P2, C2)
                    if kk < ABS_ACT_FRAC * NPAIRS:
                        a2 = big.tile([128, 2, 3, 512], DT, tag="a2")
                        nc.scalar.activation(
                            a2[:], s2[:], mybir.ActivationFunctionType.Abs
                        )
                    else:
                        nc.vector.tensor_scalar(
                            s2[:].bitcast(mybir.dt.int16),
                            s2[:].bitcast(mybir.dt.int16),
                            0x7FFF, None, mybir.AluOpType.bitwise_and,
                        )
                        a2 = s2
                    dp = dpsum.tile([128, 2, 512], mybir.dt.float32, tag="dp")
                    for p in range(2):
                        for c in range(3):
                            nc.tensor.matmul(
                                dp[:, p, :], ident[:], a2[:, p, c, :],
                                start=(c == 0), stop=(c == 2),
                                skip_group_check=True,
                            )
                    u2 = work.tile([128, 2, 512], DT, tag="u2")
                    nc.scalar.activation(
                        u2[:], dp[:], mybir.ActivationFunctionType.Square
                    )
                    w2 = work.tile([128, 2, 512], DT, tag="w2")
                    nc.scalar.activation(
                        w2[:], u2[:], mybir.ActivationFunctionType.Exp,
                        bias=biases[:, k : k + 1], scale=GAMMA,
                    )
                    t2 = big.tile([128, 2, 3, 512], DT, tag="t2")
                    w2b = w2[:].unsqueeze(2).broadcast_to([128, 2, 3, 512])
                    nc.vector.tensor_mul(t2[:], P2, w2b)
                    sp = kk == NPAIRS - 1
                    for p in range(2):
                        for c in range(3):
                            nc.tensor.matmul(
                                acc[:, c, :], ident[:], t2[:, p, c, :],
                                start=False, stop=(sp and p == 1),
                                skip_group_check=True,
                            )
                        nc.tensor.matmul(
                            den[:], ident[:], w2[:, p, :],
                            start=(kk == 0 and p == 0), stop=(sp and p == 1),
                            skip_group_check=True,
                        )

                # r = 1/(den + s24) via one Newton step from y1 = 2 - dn:
                # dn is within ~3% of 1 so the result is good to ~5e-7 rel.
                r = outp.tile([128, 512], mybir.dt.float32, tag="r")
                dn = outp.tile([128, 512], mybir.dt.float32, tag="dn")
                y1 = outp.tile([128, 512], mybir.dt.float32, tag="y1")
                e1 = outp.tile([128, 512], mybir.dt.float32, tag="e1")
                nc.vector.tensor_scalar(
                    dn[:], den[:], float(_SPACE[3, 3]), None,
                    mybir.AluOpType.add,
                )
                nc.vector.tensor_scalar(
                    y1[:], dn[:], -1.0, 2.0, mybir.AluOpType.mult,
                    mybir.AluOpType.add,
                )
                nc.vector.tensor_mul(e1[:], dn[:], y1[:])
                nc.vector.tensor_scalar(
                    e1[:], e1[:], -1.0, 2.0, mybir.AluOpType.mult,
                    mybir.AluOpType.add,
                )
                nc.vector.tensor_mul(r[:], e1[:], y1[:])
                o = outp.tile([128, 3, 512], mybir.dt.float32, tag="o")
                rb = r[:].unsqueeze(1).broadcast_to([128, 3, 512])
                nc.vector.tensor_mul(o[:], acc[:], rb)
                nc.sync.dma_start(
                    out=out_d.ap()[:, y0 : y0 + 128, :].transpose([1, 0, 2]),
                    in_=o[:],
                )

    nc.compile()
    return nc


_NC_CACHE = {}


def _get_nc():
    if "nc" not in _NC_CACHE:
        _NC_CACHE["nc"] = _build()
    return _NC_CACHE["nc"]


def _host_inputs(img_core: np.ndarray):
    p = np.pad(img_core, ((0, 0), (PAD, PAD), (PAD, PAD)), mode="reflect")
    return {
        "pad": np.ascontiguousarray(p.astype(ml_dtypes.bfloat16)),
        "ident": np.eye(128, dtype=np.float32).astype(ml_dtypes.bfloat16),
        "ident2": (np.eye(128, dtype=np.float32) * float(_SPACE[3, 3])
                   ).astype(ml_dtypes.bfloat16),
        "bias49": np.log(_SPACE.reshape(-1)).astype(np.float32),
    }


def kernel(img: np.ndarray) -> np.ndarray:
    """img: (8, 3, 512, 512) float32 -> (8, 3, 512, 512) float32."""
    img = np.asarray(img, dtype=np.float32)
    assert img.shape == (B, CH, H, W), img.shape

    nc = _get_nc()
    in_maps = [_host_inputs(img[b]) for b in range(B)]
    res = run_bass_kernel_spmd(nc, in_maps, core_ids=list(range(N_CORES)))
    out = np.stack([res.results[b]["out"] for b in range(B)], axis=0)
    return out.astype(np.float32)


# revision 5
# speedup vs baseline: 1.1457x; 1.0166x over previous
"""Bilateral blur (kornia bilateral_blur, kernel 7x7, sigma_color=10,
sigma_space=(21,21), border reflect, L1 color distance) for a batch of
8 RGB 512x512 images, on 8 Trainium2 NeuronCores.

kernel(img) takes the FULL (8, 3, 512, 512) float32 batch and returns the
FULL (8, 3, 512, 512) float32 result. The batch is sharded one image per
NeuronCore (pure data parallelism); each core runs an identical Bass/Tile
kernel built here:

  - host pads each image to (3, 518, 518) reflect and casts to bf16
  - partition dim = 128 output rows (4 row-tiles per image)
  - all 7 row-shifted copies of the padded rows live in one SBUF tile
    [128, 7, 3, 520], loaded twice at x-offsets 0/1 ("phases") so every
    window x-shift is a 4-byte-aligned bf16 slice (keeps DVE 2x mode)
  - the 49 window offsets are processed as 24 mirror PAIRS (k, 48-k) plus
    the center: mirrored offsets share the same spatial weight, so the
    whole chain runs at doubled free-dim (half the instruction overhead,
    one Exp bias per pair); the center offset has w == space[3,3] exactly
    and reduces to three identity-matmuls with a pre-scaled identity.
  - per pair:
      s   = P - C                        (DVE tensor_tensor, bf16 2x)
      a   = |s|                          (every 3rd pair on ACT Abs, rest
                                          DVE int16 sign-mask - interleaved
                                          so both engines stay fed)
      d   = a_r + a_g + a_b              (PE identity-matmuls into PSUM)
      u   = d^2 ; w = exp(g*u + ln s_k)  (ACT Square + Exp, scale/bias
                                          folded into the Exp affine)
      t   = w * P                        (DVE, w broadcast over channels)
      acc += t_0, t_1 ; den += w_0+w_1   (PE identity-matmuls into PSUM,
                                          exact fp32 accumulation)
  - epilogue: out = acc / (den + s_24)   (DVE reciprocal + multiply, fp32)
"""

import numpy as np
import ml_dtypes

import concourse.bass as bass
import concourse.bacc as bacc
import concourse.mybir as mybir
import concourse.tile as tile
from concourse.bass_utils import run_bass_kernel_spmd

KS = 7
PAD = 3
SIGMA_COLOR = 10.0
SIGMA_SPACE = 21.0
B, CH, H, W = 8, 3, 512, 512
PW = W + 2 * PAD  # 518
GAMMA = -0.5 / (SIGMA_COLOR**2)
N_CORES = 8
NPAIRS = 24


def _gauss1d(ks, sigma):
    x = np.arange(ks, dtype=np.float64) - ks // 2
    g = np.exp(-0.5 * (x / sigma) ** 2)
    return g / g.sum()


_SPACE = np.outer(_gauss1d(KS, SIGMA_SPACE), _gauss1d(KS, SIGMA_SPACE))


def _build():
    DT = mybir.dt.bfloat16
    nphase = 2

    nc = bacc.Bacc("TRN2", target_bir_lowering=False, debug=False,
                   num_devices=N_CORES)
    pad_d = nc.dram_tensor("pad", [CH, H + 2 * PAD, PW], DT,
                           kind="ExternalInput")
    id_d = nc.dram_tensor("ident", [128, 128], DT, kind="ExternalInput")
    id2_d = nc.dram_tensor("ident2", [128, 128], DT, kind="ExternalInput")
    bias_d = nc.dram_tensor("bias49", [KS * KS], mybir.dt.float32,
                            kind="ExternalInput")
    out_d = nc.dram_tensor("out", [CH, H, W], mybir.dt.float32,
                           kind="ExternalOutput")

    with tile.TileContext(nc) as tc:
        with (
            tc.tile_pool(name="consts", bufs=1) as consts,
            tc.tile_pool(name="tin", bufs=2) as tin,
            tc.tile_pool(name="work", bufs=7) as work,
            tc.tile_pool(name="big", bufs=3) as big,
            tc.tile_pool(name="outp", bufs=2) as outp,
            tc.tile_pool(name="psum", bufs=1, space="PSUM") as psum,
            tc.tile_pool(name="dpsum", bufs=2, space="PSUM") as dpsum,
        ):
            ident = consts.tile([128, 128], DT)
            nc.sync.dma_start(out=ident[:], in_=id_d.ap())
            ident2 = consts.tile([128, 128], DT)
            nc.sync.dma_start(out=ident2[:], in_=id2_d.ap())
            biases = consts.tile([128, KS * KS], mybir.dt.float32)
            bsrc = bias_d.ap()
            bsrc_b = bass.AP(
                tensor=bsrc.tensor, offset=bsrc.offset,
                ap=[[0, 128], bsrc.ap[0]],
            )
            nc.sync.dma_start(out=biases[:], in_=bsrc_b)

            for yt in range(H // 128):
                y0 = 128 * yt
                Tall = {}
                for ph in range(nphase):
                    tt = tin.tile([128, KS, 3, 520], DT, tag=f"Tall{ph}")
                    Tall[ph] = tt
                    for i in range(KS):
                        xl = PW - ph
                        src = pad_d.ap()[:, y0 + i : y0 + i + 128, ph:PW]
                        nc.sync.dma_start(
                            out=tt[:, i, :, 0:xl], in_=src.transpose([1, 0, 2])
                        )

                def pslice(i, j):
                    ph = j % 2
                    e0 = j - ph
                    return Tall[ph][:, i, :, e0 : e0 + 512]

                def pairslice(k):
                    # [128, 2, 3, 512] covering offsets k and 48-k
                    i, j = divmod(k, KS)
                    s0 = pslice(i, j)
                    s1 = pslice(6 - i, 6 - j)
                    step = s1.offset - s0.offset
                    return bass.AP(
                        tensor=s0.tensor, offset=s0.offset,
                        ap=[s0.ap[0], [step, 2], s0.ap[1], s0.ap[2]],
                    )

                C = pslice(PAD, PAD)
                C2 = C.unsqueeze(1).broadcast_to([128, 2, 3, 512])

                acc = psum.tile([128, 3, 512], mybir.dt.float32, tag="acc")
                den = psum.tile([128, 512], mybir.dt.float32, tag="den")

                # center offset: acc += space[3,3] * C via pre-scaled identity
                for c in range(3):
                    nc.tensor.matmul(
                        acc[:, c, :], ident2[:], C[:, c, :],
                        start=True, stop=False, skip_group_check=True,
                    )

                for kk in range(NPAIRS):
                    k = kk
                    P2 = pairslice(k)
                    s2 = big.tile([128, 2, 3, 512], DT, tag="s2")
                    nc.vector.tensor_sub(s2[:], P2, C2)
                    if kk % 3 == 0:  # interleave ACT-abs pairs evenly (~1/3)
                        a2 = big.tile([128, 2, 3, 512], DT, tag="a2")
                        nc.scalar.activation(
                            a2[:], s2[:], mybir.ActivationFunctionType.Abs
                        )
                    else:
                        nc.vector.tensor_scalar(
                            s2[:].bitcast(mybir.dt.int16),
                            s2[:].bitcast(mybir.dt.int16),
                            0x7FFF, None, mybir.AluOpType.bitwise_and,
                        )
                        a2 = s2
                    dp = dpsum.tile([128, 2, 512], mybir.dt.float32, tag="dp")
                    for p in range(2):
                        for c in range(3):
                            nc.tensor.matmul(
                                dp[:, p, :], ident[:], a2[:, p, c, :],
                                start=(c == 0), stop=(c == 2),
                                skip_group_check=True,
                            )
                    u2 = work.tile([128, 2, 512], DT, tag="u2")
                    nc.scalar.activation(
                        u2[:], dp[:], mybir.ActivationFunctionType.Square
                    )
                    w2 = work.tile([128, 2, 512], DT, tag="w2")
                    nc.scalar.activation(
                        w2[:], u2[:], mybir.ActivationFunctionType.Exp,
                        bias=biases[:, k : k + 1], scale=GAMMA,
                    )
                    t2 = big.tile([128, 2, 3, 512], DT, tag="t2")
                    w2b = w2[:].unsqueeze(2).broadcast_to([128, 2, 3, 512])
                    nc.vector.tensor_mul(t2[:], P2, w2b)
                    sp = kk == NPAIRS - 1
                    for p in range(2):
                        for c in range(3):
                            nc.tensor.matmul(
                                acc[:, c, :], ident[:], t2[:, p, c, :],
                                start=False, stop=(sp and p == 1),
                                skip_group_check=True,
                            )
                        nc.tensor.matmul(
                            den[:], ident[:], w2[:, p, :],
                            start=(kk == 0 and p == 0), stop=(sp and p == 1),
                            skip_group_check=True,
                        )

                # r = 1/(den + s24) via one Newton step from y1 = 2 - dn:
                # dn is within ~3% of 1 so the result is good to ~5e-7 rel.
                r = outp.tile([128, 512], mybir.dt.float32, tag="r")
                dn = outp.tile([128, 512], mybir.dt.float32, tag="dn")
                y1 = outp.tile([128, 512], mybir.dt.float32, tag="y1")
                e1 = outp.tile([128, 512], mybir.dt.float32, tag="e1")
                nc.vector.tensor_scalar(
                    dn[:], den[:], float(_SPACE[3, 3]), None,
                    mybir.AluOpType.add,
                )
                nc.vector.tensor_scalar(
                    y1[:], dn[:], -1.0, 2.0, mybir.AluOpType.mult,
                    mybir.AluOpType.add,
                )
                nc.vector.tensor_mul(e1[:], dn[:], y1[:])
                nc.vector.tensor_scalar(
                    e1[:], e1[:], -1.0, 2.0, mybir.AluOpType.mult,
                    mybir.AluOpType.add,
                )
                nc.vector.tensor_mul(r[:], e1[:], y1[:])
                o = outp.tile([128, 3, 512], mybir.dt.float32, tag="o")
                rb = r[:].unsqueeze(1).broadcast_to([128, 3, 512])
                nc.vector.tensor_mul(o[:], acc[:], rb)
                nc.sync.dma_start(
                    out=out_d.ap()[:, y0 : y0 + 128, :].transpose([1, 0, 2]),
                    in_=o[:],
                )

    nc.compile()
    return nc


_NC_CACHE = {}


def _get_nc():
    if "nc" not in _NC_CACHE:
        _NC_CACHE["nc"] = _build()
    return _NC_CACHE["nc"]


def _host_inputs(img_core: np.ndarray):
    p = np.pad(img_core, ((0, 0), (PAD, PAD), (PAD, PAD)), mode="reflect")
    return {
        "pad": np.ascontiguousarray(p.astype(ml_dtypes.bfloat16)),
        "ident": np.eye(128, dtype=np.float32).astype(ml_dtypes.bfloat16),
        "ident2": (np.eye(128, dtype=np.float32) * float(_SPACE[3, 3])
                   ).astype(ml_dtypes.bfloat16),
        "bias49": np.log(_SPACE.reshape(-1)).astype(np.float32),
    }


def kernel(img: np.ndarray) -> np.ndarray:
    """img: (8, 3, 512, 512) float32 -> (8, 3, 512, 512) float32."""
    img = np.asarray(img, dtype=np.float32)
    assert img.shape == (B, CH, H, W), img.shape

    nc = _get_nc()
    in_maps = [_host_inputs(img[b]) for b in range(B)]
    res = run_bass_kernel_spmd(nc, in_maps, core_ids=list(range(N_CORES)))
    out = np.stack([res.results[b]["out"] for b in range(B)], axis=0)
    return out.astype(np.float32)


# revision 6
# speedup vs baseline: 1.2174x; 1.0626x over previous
"""Bilateral blur (kornia bilateral_blur, kernel 7x7, sigma_color=10,
sigma_space=(21,21), border reflect, L1 color distance) for a batch of
8 RGB 512x512 images, on 8 Trainium2 NeuronCores.

kernel(img) takes the FULL (8, 3, 512, 512) float32 batch and returns the
FULL (8, 3, 512, 512) float32 result. The batch is sharded one image per
NeuronCore (pure data parallelism); each core runs an identical Bass/Tile
kernel built here:

  - host pads each image to (3, 518, 518) reflect and casts to bf16
  - partition dim = 128 output rows (4 row-tiles per image)
  - all 7 row-shifted copies of the padded rows live in one SBUF tile
    [128, 7, 3, 520], loaded twice at x-offsets 0/1 ("phases") so every
    window x-shift is a 4-byte-aligned bf16 slice (keeps DVE 2x mode)
  - the 49 window offsets are processed as 24 mirror PAIRS (k, 48-k) plus
    the center: mirrored offsets share the same spatial weight, so the
    whole chain runs at doubled free-dim (half the instruction overhead,
    one Exp bias per pair); the center offset has w == space[3,3] exactly
    and reduces to three identity-matmuls with a pre-scaled identity.
  - per pair:
      s   = P - C                        (DVE tensor_tensor, bf16 2x)
      a   = |s|                          (every 2nd pair on ACT Abs, rest
                                          DVE int16 sign-mask - interleaved
                                          so both engines stay fed)
      d   = a_r + a_g + a_b              (PE identity-matmuls into PSUM)
      u   = d^2 ; w = exp(g*u + ln s_k)  (ACT Square + Exp, scale/bias
                                          folded into the Exp affine)
      t   = w * P                        (DVE, w broadcast over channels)
      acc += t_0, t_1 ; den += w_0+w_1   (PE identity-matmuls into PSUM,
                                          exact fp32 accumulation)
  - epilogue: out = acc / (den + s_24)   (DVE reciprocal + multiply, fp32)
"""

import numpy as np
import ml_dtypes

import concourse.bass as bass
import concourse.bacc as bacc
import concourse.mybir as mybir
import concourse.tile as tile
from concourse.bass_utils import run_bass_kernel_spmd

KS = 7
PAD = 3
SIGMA_COLOR = 10.0
SIGMA_SPACE = 21.0
B, CH, H, W = 8, 3, 512, 512
PW = W + 2 * PAD  # 518
GAMMA = -0.5 / (SIGMA_COLOR**2)
N_CORES = 8
NPAIRS = 24


def _gauss1d(ks, sigma):
    x = np.arange(ks, dtype=np.float64) - ks // 2
    g = np.exp(-0.5 * (x / sigma) ** 2)
    return g / g.sum()


_SPACE = np.outer(_gauss1d(KS, SIGMA_SPACE), _gauss1d(KS, SIGMA_SPACE))


def _build():
    DT = mybir.dt.bfloat16
    nphase = 2

    nc = bacc.Bacc("TRN2", target_bir_lowering=False, debug=False,
                   num_devices=N_CORES)
    pad_d = nc.dram_tensor("pad", [CH, H + 2 * PAD, PW], DT,
                           kind="ExternalInput")
    id_d = nc.dram_tensor("ident", [128, 128], DT, kind="ExternalInput")
    id2_d = nc.dram_tensor("ident2", [128, 128], DT, kind="ExternalInput")
    bias_d = nc.dram_tensor("bias49", [KS * KS], mybir.dt.float32,
                            kind="ExternalInput")
    out_d = nc.dram_tensor("out", [CH, H, W], mybir.dt.float32,
                           kind="ExternalOutput")

    with tile.TileContext(nc) as tc:
        with (
            tc.tile_pool(name="consts", bufs=1) as consts,
            tc.tile_pool(name="tin", bufs=2) as tin,
            tc.tile_pool(name="work", bufs=7) as work,
            tc.tile_pool(name="big", bufs=3) as big,
            tc.tile_pool(name="outp", bufs=2) as outp,
            tc.tile_pool(name="psum", bufs=1, space="PSUM") as psum,
            tc.tile_pool(name="dpsum", bufs=2, space="PSUM") as dpsum,
        ):
            ident = consts.tile([128, 128], DT)
            nc.sync.dma_start(out=ident[:], in_=id_d.ap())
            ident2 = consts.tile([128, 128], DT)
            nc.sync.dma_start(out=ident2[:], in_=id2_d.ap())
            biases = consts.tile([128, KS * KS], mybir.dt.float32)
            bsrc = bias_d.ap()
            bsrc_b = bass.AP(
                tensor=bsrc.tensor, offset=bsrc.offset,
                ap=[[0, 128], bsrc.ap[0]],
            )
            nc.sync.dma_start(out=biases[:], in_=bsrc_b)

            for yt in range(H // 128):
                y0 = 128 * yt
                Tall = {}
                for ph in range(nphase):
                    tt = tin.tile([128, KS, 3, 520], DT, tag=f"Tall{ph}")
                    Tall[ph] = tt
                    for i in range(KS):
                        xl = PW - ph
                        src = pad_d.ap()[:, y0 + i : y0 + i + 128, ph:PW]
                        nc.sync.dma_start(
                            out=tt[:, i, :, 0:xl], in_=src.transpose([1, 0, 2])
                        )

                def pslice(i, j):
                    ph = j % 2
                    e0 = j - ph
                    return Tall[ph][:, i, :, e0 : e0 + 512]

                def pairslice(k):
                    # [128, 2, 3, 512] covering offsets k and 48-k
                    i, j = divmod(k, KS)
                    s0 = pslice(i, j)
                    s1 = pslice(6 - i, 6 - j)
                    step = s1.offset - s0.offset
                    return bass.AP(
                        tensor=s0.tensor, offset=s0.offset,
                        ap=[s0.ap[0], [step, 2], s0.ap[1], s0.ap[2]],
                    )

                C = pslice(PAD, PAD)
                C2 = C.unsqueeze(1).broadcast_to([128, 2, 3, 512])

                acc = psum.tile([128, 3, 512], mybir.dt.float32, tag="acc")
                den = psum.tile([128, 512], mybir.dt.float32, tag="den")

                # center offset: acc += space[3,3] * C via pre-scaled identity
                for c in range(3):
                    nc.tensor.matmul(
                        acc[:, c, :], ident2[:], C[:, c, :],
                        start=True, stop=False, skip_group_check=True,
                    )

                for kk in range(NPAIRS):
                    k = kk
                    P2 = pairslice(k)
                    s2 = big.tile([128, 2, 3, 512], DT, tag="s2")
                    nc.vector.tensor_sub(s2[:], P2, C2)
                    if kk % 2 == 0:  # interleave ACT-abs pairs evenly (1/2)
                        a2 = big.tile([128, 2, 3, 512], DT, tag="a2")
                        nc.scalar.activation(
                            a2[:], s2[:], mybir.ActivationFunctionType.Abs
                        )
                    else:
                        nc.vector.tensor_scalar(
                            s2[:].bitcast(mybir.dt.int16),
                            s2[:].bitcast(mybir.dt.int16),
                            0x7FFF, None, mybir.AluOpType.bitwise_and,
                        )
                        a2 = s2
                    dp = dpsum.tile([128, 2, 512], mybir.dt.float32, tag="dp")
                    for p in range(2):
                        for c in range(3):
                            nc.tensor.matmul(
                                dp[:, p, :], ident[:], a2[:, p, c, :],
                                start=(c == 0), stop=(c == 2),
                                skip_group_check=True,
                            )
                    u2 = work.tile([128, 2, 512], DT, tag="u2")
                    nc.scalar.activation(
                        u2[:], dp[:], mybir.ActivationFunctionType.Square
                    )
                    w2 = work.tile([128, 2, 512], DT, tag="w2")
                    nc.scalar.activation(
                        w2[:], u2[:], mybir.ActivationFunctionType.Exp,
                        bias=biases[:, k : k + 1], scale=GAMMA,
                    )
                    t2 = big.tile([128, 2, 3, 512], DT, tag="t2")
                    w2b = w2[:].unsqueeze(2).broadcast_to([128, 2, 3, 512])
                    nc.vector.tensor_mul(t2[:], P2, w2b)
                    sp = kk == NPAIRS - 1
                    for p in range(2):
                        for c in range(3):
                            nc.tensor.matmul(
                                acc[:, c, :], ident[:], t2[:, p, c, :],
                                start=False, stop=(sp and p == 1),
                                skip_group_check=True,
                            )
                        nc.tensor.matmul(
                            den[:], ident[:], w2[:, p, :],
                            start=(kk == 0 and p == 0), stop=(sp and p == 1),
                            skip_group_check=True,
                        )

                # r = 1/(den + s24) via one Newton step from y1 = 2 - dn:
                # dn is within ~3% of 1 so the result is good to ~5e-7 rel.
                r = outp.tile([128, 512], mybir.dt.float32, tag="r")
                dn = outp.tile([128, 512], mybir.dt.float32, tag="dn")
                y1 = outp.tile([128, 512], mybir.dt.float32, tag="y1")
                e1 = outp.tile([128, 512], mybir.dt.float32, tag="e1")
                nc.vector.tensor_scalar(
                    dn[:], den[:], float(_SPACE[3, 3]), None,
                    mybir.AluOpType.add,
                )
                nc.vector.tensor_scalar(
                    y1[:], dn[:], -1.0, 2.0, mybir.AluOpType.mult,
                    mybir.AluOpType.add,
                )
                nc.vector.tensor_mul(e1[:], dn[:], y1[:])
                nc.vector.tensor_scalar(
                    e1[:], e1[:], -1.0, 2.0, mybir.AluOpType.mult,
                    mybir.AluOpType.add,
                )
                nc.vector.tensor_mul(r[:], e1[:], y1[:])
                o = outp.tile([128, 3, 512], mybir.dt.float32, tag="o")
                rb = r[:].unsqueeze(1).broadcast_to([128, 3, 512])
                nc.vector.tensor_mul(o[:], acc[:], rb)
                nc.sync.dma_start(
                    out=out_d.ap()[:, y0 : y0 + 128, :].transpose([1, 0, 2]),
                    in_=o[:],
                )

    nc.compile()
    return nc


_NC_CACHE = {}


def _get_nc():
    if "nc" not in _NC_CACHE:
        _NC_CACHE["nc"] = _build()
    return _NC_CACHE["nc"]


def _host_inputs(img_core: np.ndarray):
    p = np.pad(img_core, ((0, 0), (PAD, PAD), (PAD, PAD)), mode="reflect")
    return {
        "pad": np.ascontiguousarray(p.astype(ml_dtypes.bfloat16)),
        "ident": np.eye(128, dtype=np.float32).astype(ml_dtypes.bfloat16),
        "ident2": (np.eye(128, dtype=np.float32) * float(_SPACE[3, 3])
                   ).astype(ml_dtypes.bfloat16),
        "bias49": np.log(_SPACE.reshape(-1)).astype(np.float32),
    }


def kernel(img: np.ndarray) -> np.ndarray:
    """img: (8, 3, 512, 512) float32 -> (8, 3, 512, 512) float32."""
    img = np.asarray(img, dtype=np.float32)
    assert img.shape == (B, CH, H, W), img.shape

    nc = _get_nc()
    in_maps = [_host_inputs(img[b]) for b in range(B)]
    res = run_bass_kernel_spmd(nc, in_maps, core_ids=list(range(N_CORES)))
    out = np.stack([res.results[b]["out"] for b in range(B)], axis=0)
    return out.astype(np.float32)
